# revision 29
# baseline (speedup 1.0000x reference)
"""Trainium2 Bass kernel for kornia-style 3x3 grayscale dilation.

Problem: img (64,1,1024,1024) f32, kernel 3x3 ones.
out[y,x] = max over 3x3 neighborhood of img padded with -1e4 (geodesic border).

Measured-on-HW design notes:
  - DRAM contiguity dominates DMA rate (~60-83 GB/s for 1KB strided chunks vs
    ~357 GB/s contiguous): the host re-lays the input into per-(tile,
    partition) contiguous (R+2)x(C+2) blocks (halos pre-built), and the
    OUTPUT is written band-major as contiguous per-partition blocks that the
    host de-interleaves afterwards.
  - DVE multi-row APs cost ~1.3-1.7us per extra outer-dim row, so every
    compute op is a single flat 1-D tensor_tensor max (junk at the row seams
    is carried through and sliced off on the host).
  - fp32 TT max on DVE ~0.6-1.0 ns/elem at stride 1 (stride 2 is 2.3x worse,
    GPSIMD 3-4 ns/elem) -> plain 2-pass separable max, all on DVE.
All max ops are native f32 max -> results are bit-exact vs the reference.

Sharding: pure data parallel, 8 images per core (batch dim).
"""

import numpy as np

MAX_VAL = 1e4

# ---------------------------------------------------------------------------
N_CORES = 8
B_PER_CORE = 8
H = 1024
W = 1024
R = 8               # rows per partition chunk (even)
C = 512             # band width in cols
T_BUFS, M_BUFS, V_BUFS, Q_BUFS = 3, 1, 2, 2


def _geom(B, H, W, R, C):
    G = B * H
    cpi = H // R                # chunks per image
    assert H % R == 0 and cpi <= 128 and 128 % cpi == 0
    ipt = 128 // cpi            # images per tile group
    n_tg = B // ipt
    assert B % ipt == 0
    n_bands = W // C
    assert W % C == 0 and R % 2 == 0 and C % 2 == 0
    return G, cpi, ipt, n_tg, n_bands


def _owidth(R, C):
    """Per-partition elements in one output block (R rows at stride C+2,
    last row only C+1 wide -> R*(C+2)-2 covers through col C-1 of row R-1)."""
    return R * (C + 2) - 2


def build_dilation_program(B=B_PER_CORE, H=H, W=W, R=R, C=C,
                           t_bufs=T_BUFS, m_bufs=M_BUFS, v_bufs=V_BUFS,
                           q_bufs=Q_BUFS):
    import concourse.bacc as bacc
    import concourse.mybir as mybir
    import concourse.tile as tile
    from concourse.ap import AP
    from contextlib import ExitStack

    f32 = mybir.dt.float32
    MAX = mybir.AluOpType.max
    G, cpi, ipt, n_tg, n_bands = _geom(B, H, W, R, C)
    n_tiles = n_tg * n_bands

    CW = C + 2
    TROW = CW
    T_W = (R + 2) * TROW        # contiguous input block per partition
    M_W = (R + 1) * CW          # vertical pair-max (rows at stride CW)
    V_W = R * CW                # vertical 3-max (rows at stride CW)
    Q_W = R * CW + 4            # horizontal tmp/result (rows at stride CW)
    O_W = _owidth(R, C)         # stored span per partition (flat, with seams)

    nc = bacc.Bacc("TRN2", target_bir_lowering=False, debug=False)
    img_h = nc.declare_dram_parameter("img", [n_tiles * 128, T_W], f32,
                                      isOutput=False)
    out_h = nc.declare_dram_parameter("out", [n_tiles * 128, O_W], f32,
                                      isOutput=True)
    img = img_h[:]
    outp = out_h[:]

    def sub(t, p0, pc, foff, fd):
        ps = t.ap[0][0]
        return AP(t.tensor, t.offset + p0 * ps + foff, [[ps, pc]] + list(fd))

    with ExitStack() as ctx:
        tc = ctx.enter_context(tile.TileContext(nc))
        t_pool = ctx.enter_context(tc.tile_pool(name="t", bufs=t_bufs))
        m_pool = ctx.enter_context(tc.tile_pool(name="m", bufs=m_bufs))
        v_pool = ctx.enter_context(tc.tile_pool(name="v", bufs=v_bufs))
        q_pool = ctx.enter_context(tc.tile_pool(name="q", bufs=q_bufs))

        for ti in range(n_tiles):
            T = t_pool.tile([128, T_W], f32, name="T", tag="T")
            M = m_pool.tile([128, M_W], f32, name="M", tag="M")
            V = v_pool.tile([128, V_W], f32, name="V", tag="V")
            Q = q_pool.tile([128, Q_W], f32, name="Q", tag="Q")

            # ---- load: one DMA, contiguous per partition -------------------
            nc.sync.dma_start(
                out=sub(T, 0, 128, 0, [[1, T_W]]),
                in_=AP(img.tensor, ti * 128 * T_W, [[T_W, 128], [1, T_W]]),
            )

            # ---- all-flat compute (rows share stride CW) -------------------
            # M[r] = max(T[r], T[r+1]), r = 0..R   (flat across rows)
            nc.vector.tensor_tensor(
                out=sub(M, 0, 128, 0, [[1, M_W]]),
                in0=sub(T, 0, 128, 0, [[1, M_W]]),
                in1=sub(T, 0, 128, TROW, [[1, M_W]]),
                op=MAX,
            )
            # V[r] = max(M[r], T[r+2]), r = 0..R-1  -> vertical 3-max
            nc.vector.tensor_tensor(
                out=sub(V, 0, 128, 0, [[1, V_W]]),
                in0=sub(M, 0, 128, 0, [[1, V_W]]),
                in1=sub(T, 0, 128, 2 * TROW, [[1, V_W]]),
                op=MAX,
            )
            # Q[x] = max(V[x], V[x+1])  (pair max; junk at row seams)
            nc.vector.tensor_tensor(
                out=sub(Q, 0, 128, 0, [[1, V_W - 1]]),
                in0=sub(V, 0, 128, 0, [[1, V_W - 1]]),
                in1=sub(V, 0, 128, 1, [[1, V_W - 1]]),
                op=MAX,
            )
            # Q[x] = max(Q[x], V[x+2]) in-place -> horizontal 3-max
            nc.vector.tensor_tensor(
                out=sub(Q, 0, 128, 0, [[1, V_W - 2]]),
                in0=sub(Q, 0, 128, 0, [[1, V_W - 2]]),
                in1=sub(V, 0, 128, 2, [[1, V_W - 2]]),
                op=MAX,
            )

            # ---- store: contiguous per-partition block (seams included) ----
            nc.scalar.dma_start(
                out=AP(outp.tensor, ti * 128 * O_W, [[O_W, 128], [1, O_W]]),
                in_=sub(Q, 0, 128, 0, [[1, O_W]]),
            )

    nc.finalize()
    return nc


def make_blocks(flat, B=B_PER_CORE, Himg=H, Wimg=W, R=R, C=C):
    """Relayout one core's stacked images (B*Himg, Wimg) into contiguous
    per-(tile, partition) blocks of (R+2)x(C+2) incl. -1e4 halos."""
    G, cpi, ipt, n_tg, n_bands = _geom(B, Himg, Wimg, R, C)
    pad = np.full((B, Himg + 2, Wimg + 2), np.float32(-MAX_VAL), np.float32)
    pad[:, 1:-1, 1:-1] = flat.reshape(B, Himg, Wimg)
    sw = np.lib.stride_tricks.sliding_window_view(pad, (R + 2, C + 2),
                                                  axis=(1, 2))
    blk = sw[:, ::R, ::C]                       # [B, cpi, n_bands, R+2, C+2]
    blk = blk.reshape(n_tg, ipt, cpi, n_bands, R + 2, C + 2)
    blk = blk.transpose(0, 3, 1, 2, 4, 5)       # [n_tg, band, ipt, cpi, ...]
    return np.ascontiguousarray(blk).reshape(n_tg * n_bands * 128,
                                             (R + 2) * (C + 2))


def unblock(raw, B=B_PER_CORE, Himg=H, Wimg=W, R=R, C=C):
    """Inverse of the output blocking: raw [n_tiles*128, O_W] -> (G, W)."""
    G, cpi, ipt, n_tg, n_bands = _geom(B, Himg, Wimg, R, C)
    O_W = _owidth(R, C)
    CW = C + 2
    a = np.ascontiguousarray(raw).reshape(n_tg, n_bands, 128, O_W)
    s = a.strides
    rows = np.lib.stride_tricks.as_strided(
        a, shape=(n_tg, n_bands, 128, R, C),
        strides=(s[0], s[1], s[2], CW * 4, 4))
    # -> (n_tg, ipt, cpi, R, n_bands, C) -> rows
    rows = rows.transpose(0, 2, 3, 1, 4).reshape(n_tg, ipt, cpi, R,
                                                 n_bands * C)
    return np.ascontiguousarray(rows).reshape(G, Wimg)


# ---------------------------------------------------------------------------
_PROGRAM_CACHE = {}


def _get_program():
    key = (B_PER_CORE, H, W, R, C)
    if key not in _PROGRAM_CACHE:
        _PROGRAM_CACHE[key] = build_dilation_program()
    return _PROGRAM_CACHE[key]


def _dilation_numpy(img, kernel):
    """Exact reference semantics fallback (general 0/1 kernel)."""
    B, Ch, Hh, Ww = img.shape
    nb = np.where(kernel == 0, np.float32(-MAX_VAL), np.float32(0.0))
    nb = nb[::-1, ::-1]
    p = np.pad(img, ((0, 0), (0, 0), (1, 1), (1, 1)),
               constant_values=np.float32(-MAX_VAL))
    out = p[:, :, 0:Hh, 0:Ww] + nb[0, 0]
    for i in range(3):
        for j in range(3):
            if i == 0 and j == 0:
                continue
            np.maximum(out, p[:, :, i:i + Hh, j:j + Ww] + nb[i, j], out=out)
    return out.astype(np.float32)


def kernel(img, kernel):
    img = np.asarray(img, dtype=np.float32)
    k = np.asarray(kernel, dtype=np.float32)
    if img.shape != (64, 1, 1024, 1024) or not np.all(k == 1.0):
        return _dilation_numpy(img, k)

    from concourse.bass_utils import run_bass_kernel_spmd

    nc = _get_program()
    flat = img.reshape(N_CORES, B_PER_CORE * H, W)
    in_maps = [{"img": make_blocks(flat[c])} for c in range(N_CORES)]
    res = run_bass_kernel_spmd(nc, in_maps, list(range(N_CORES))).results
    out = np.stack([unblock(res[c]["out"]) for c in range(N_CORES)])
    return out.reshape(64, 1, 1024, 1024)


if __name__ == "__main__":
    rng = np.random.default_rng(0)
    a = rng.random((2, 1, 8, 8), dtype=np.float32)
    k = np.ones((3, 3), np.float32)
    print(_dilation_numpy(a, k)[0, 0, :3, :3])
